# revision 1
# baseline (speedup 1.0000x reference)
"""Trainium2 Bass kernel for DistanceBasedAttention (L1-distance attention).

Contract: kernel(**inputs) takes FULL unsharded inputs (as produced by
setup_inputs()) and returns the FULL output [B, S, HID] float32.

Sharding: the 16 (batch, head) blocks are split 2-per-core across 8 cores
(core = b*4 + head_pair). Each core computes its two heads' attention output
and the partial out-projection (Wo rows of its heads); the host sums the four
per-batch partials and adds the effective bias (bv @ Wo + bo).

Math trick used on-device: with S = LAMBDA/sqrt(HD),
    D[j,i] = sum_d |q_id - k_jd| = 2*sum_d relu(q_id - k_jd) - Qsum[i] + Ksum[j]
The -Qsum[i] term is constant along the softmax axis (j) and cancels in
softmax, so it is never computed. relu tiles are produced by DVE
tensor_scalar(add, max) / ScalarE Relu activation, reduced over d by
TensorEngine matmuls with 0/2 selector weights (4 column-groups running
concurrently), and Ksum[j] is added with one rank-1 matmul per 32-row group.
"""

import numpy as np

import concourse.bass as bass
import concourse.mybir as mybir
import concourse.tile as tile
from concourse.bass_utils import run_bass_kernel_spmd

F32 = mybir.dt.float32
F16 = mybir.dt.float16
Alu = mybir.AluOpType
Act = mybir.ActivationFunctionType

B, S, HID = 2, 512, 512
NH, HD = 8, 64
LAMBDA = 1.0
SCALE = float(LAMBDA / np.sqrt(HD))
N_CORES = 8

# fraction of A-tiles handled by ScalarE (rest on DVE): ACT_N out of each ACT_D
import os as _os
ACT_N = int(_os.environ.get("DBA_ACT_N", "29"))
ACT_D = 100
WORK_BUFS = int(_os.environ.get("DBA_WORK_BUFS", "12"))
PBANK_BUFS = int(_os.environ.get("DBA_PBANK_BUFS", "4"))
SKIP = _os.environ.get("DBA_SKIP", "")
ACT_PAT = _os.environ.get("DBA_ACT_PAT", "counter")  # "", "mm", "elem" (diagnostics only)


def _split_excess_waits(nc, max_waits=1):
    """walrus in this container accepts a single sync-wait per instruction;
    move excess waits onto same-engine NoOps inserted just before."""
    f = nc.m.functions[0]
    for bb in f.blocks:
        new_list = []
        changed = False
        for ins in bb.instructions:
            si = ins.sync_info
            if si is not None and si.on_wait is not None and len(si.on_wait) > max_waits:
                waits = list(si.on_wait)
                k = 0
                while len(waits) - k > max_waits:
                    chunk = waits[k : k + max_waits]
                    k += max_waits
                    nop = mybir.InstNoOp(name=f"{ins.name}-ws-{k}", ins=[], outs=[])
                    nop.engine = ins.engine
                    nop.sync_info = mybir.SyncInfo(on_wait=chunk, on_update=[])
                    new_list.append(nop)
                si.on_wait = waits[k:]
                changed = True
            new_list.append(ins)
        if changed:
            bb.instructions = new_list


def _build_program(repeat=0):
    nc = bass.Bass()
    hidt_d = nc.dram_tensor("hidt", [HID, S], F16, kind="ExternalInput")
    wq2_d = nc.dram_tensor("wq2", [HID, 256], F16, kind="ExternalInput")
    wk2n_d = nc.dram_tensor("wk2n", [HID, 128], F16, kind="ExternalInput")
    wv2_d = nc.dram_tensor("wv2", [HID, 128], F16, kind="ExternalInput")
    wo2_d = nc.dram_tensor("wo2", [128, HID], F16, kind="ExternalInput")
    wks2_d = nc.dram_tensor("wks2", [HID, 2], F16, kind="ExternalInput")
    bqcol_d = nc.dram_tensor("bqcol", [128, 2], F32, kind="ExternalInput")
    bkncol_d = nc.dram_tensor("bkncol", [128, 2], F32, kind="ExternalInput")
    bks2_d = nc.dram_tensor("bks2", [1, 2], F32, kind="ExternalInput")
    sel_d = nc.dram_tensor("sel", [128, 16, 32], F16, kind="ExternalInput")
    ones64_d = nc.dram_tensor("ones64", [128, 64], F16, kind="ExternalInput")
    outp_d = nc.dram_tensor("outp", [S, HID], F32, kind="ExternalOutput")

    with tile.TileContext(nc) as tc:
        with (
            tc.tile_pool(name="consts", bufs=1) as consts,
            tc.tile_pool(name="work", bufs=WORK_BUFS) as work,
            tc.tile_pool(name="pbank", bufs=PBANK_BUFS, space="PSUM") as pbank,
            tc.tile_pool(name="paux", bufs=2, space="PSUM") as paux,
        ):
            if repeat:
                import contextlib
                loop_cm = tc.For_i(
                    0, repeat, 1,
                    hint_engines=(
                        mybir.EngineType.DVE,
                        mybir.EngineType.Activation,
                        mybir.EngineType.PE,
                        mybir.EngineType.SP,
                    ),
                )
            else:
                import contextlib
                loop_cm = contextlib.nullcontext()
            with loop_cm:
                _emit_body(
                    nc, consts, work, pbank, paux,
                    hidt_d, wq2_d, wk2n_d, wv2_d, wo2_d, wks2_d,
                    bqcol_d, bkncol_d, bks2_d, sel_d, ones64_d, outp_d,
                )

    _split_excess_waits(nc)
    return nc


def _emit_body(
    nc, consts, work, pbank, paux,
    hidt_d, wq2_d, wk2n_d, wv2_d, wo2_d, wks2_d,
    bqcol_d, bkncol_d, bks2_d, sel_d, ones64_d, outp_d,
):
    # ---- DMAs: transposed fp16 hidden first (it gates the whole pipeline) ----
    hidT = consts.tile([128, 4, 512], F16, name="hidT")
    for kt in range(4):
        nc.sync.dma_start(hidT[:, kt, :], hidt_d[kt * 128 : (kt + 1) * 128, :])
    wq2 = consts.tile([128, 4, 256], F16, name="wq2")
    wk2n = consts.tile([128, 4, 128], F16, name="wk2n")
    wv2 = consts.tile([128, 4, 128], F16, name="wv2")
    for kt in range(4):
        nc.sync.dma_start(wq2[:, kt, :], wq2_d[kt * 128 : (kt + 1) * 128, :])
        nc.sync.dma_start(wk2n[:, kt, :], wk2n_d[kt * 128 : (kt + 1) * 128, :])
    sel = consts.tile([128, 16, 32], F16, name="sel")
    nc.sync.dma_start(sel[:], sel_d[:])
    bqcol = consts.tile([128, 2], F32, name="bqcol")
    nc.sync.dma_start(bqcol[:], bqcol_d[:])
    bkncol = consts.tile([128, 2], F32, name="bkncol")
    nc.sync.dma_start(bkncol[:], bkncol_d[:])
    bks2 = consts.tile([1, 2], F32, name="bks2")
    nc.sync.dma_start(bks2[:], bks2_d[:])
    wks2 = consts.tile([128, 4, 2], F16, name="wks2")
    for kt in range(4):
        nc.sync.dma_start(wks2[:, kt, :], wks2_d[kt * 128 : (kt + 1) * 128, :])
    for kt in range(4):
        nc.sync.dma_start(wv2[:, kt, :], wv2_d[kt * 128 : (kt + 1) * 128, :])
    ones64 = consts.tile([128, 64], F16, name="ones64")
    nc.sync.dma_start(ones64[:], ones64_d[:])
    onesrow = consts.tile([1, 512], F32, name="onesrow")
    nc.any.memset(onesrow[:], 1.0)
    wo2 = consts.tile([128, 512], F16, name="wo2")
    nc.sync.dma_start(wo2[:], wo2_d[:])

    def hidT_par(kt, par):
        return hidT[:, kt].rearrange("p (j two) -> p two j", two=2)[:, par, :]

    # ---- Q^T / -K^T per head (head 0 copies on DVE: ACT still idle-ish) ----
    qt2, ktp = [], []

    def emit_qkt(h):
        q_ps = pbank.tile([128, 512], F32, name="q_ps", tag="bank")
        for kt in range(4):
            nc.tensor.matmul(
                q_ps[:],
                wq2[:, kt, 128 * h : 128 * h + 128],
                hidT[:, kt, :],
                start=(kt == 0), stop=(kt == 3),
            )
        q_sb = consts.tile([128, 512], F16, name=f"qt2_{h}")
        if h == 0:
            nc.vector.tensor_scalar(
                q_sb[:], q_ps[:], bqcol[:, h : h + 1], None, Alu.add
            )
        else:
            nc.scalar.activation(
                q_sb[:], q_ps[:], Act.Identity, bias=bqcol[:, h : h + 1], scale=1.0
            )
        qt2.append(q_sb)

        k_ps = pbank.tile([128, 256], F32, name="k_ps", tag="bank")
        for par in range(2):
            for kt in range(4):
                nc.tensor.matmul(
                    k_ps[64 * par : 64 * par + 64, :],
                    wk2n[:, kt, 64 * h : 64 * h + 64],
                    hidT_par(kt, par),
                    start=(kt == 0), stop=(kt == 3),
                    tile_position=(0, 64 * par),
                )
        k_sb = consts.tile([128, 256], F32, name=f"ktp_{h}")
        # ktp copy on ACT for BOTH heads: runs concurrently with the DVE qt2
        # copy in the prologue, so neither engine's first A-tile waits on both.
        nc.scalar.activation(
            k_sb[:], k_ps[:], Act.Identity, bias=bkncol[:, h : h + 1], scale=1.0
        )
        ktp.append(k_sb)

    emit_qkt(0)

    # ---- Ksum rows (emitted after qkt0 so the gating qt2 copy leads the
    # DVE queue; corrections only need these ~12us later) ----
    ksr = []
    for h in range(2):
        ks_ps = paux.tile([1, 512], F32, name="ks_ps", tag="aux")
        for kt in range(4):
            nc.tensor.matmul(
                ks_ps[:], wks2[:, kt, h : h + 1], hidT[:, kt, :],
                start=(kt == 0), stop=(kt == 3),
            )
        k_h = consts.tile([1, 512], F32, name=f"ksr{h}")
        nc.vector.tensor_scalar(
            k_h[:], ks_ps[:], bks2[:, h : h + 1], None, Alu.add
        )
        ksr.append(k_h)

    # ---- distance banks ----
    et_sb = [consts.tile([128, 4, 512], F16, name=f"et_{h}") for h in range(2)]
    tile_state = {"ctr": 0}
    pending_exp = []

    def flush_exp():
        while pending_exp:
            ph, pbk, pdt = pending_exp.pop(0)
            nc.scalar.activation(et_sb[ph][:, pbk, :], pdt[:], Act.Exp, scale=-SCALE)

    def emit_bank(h, bk):
        dt_ps = pbank.tile([128, 512], F32, name="dt_ps", tag="bank")
        a_static = None
        if SKIP == "elem":
            a_static = work.tile([128, 512], F16, name="a", tag="a")
            nc.vector.tensor_scalar(
                a_static[:], qt2[h][:], ktp[h][:, 0:1], 0.0, Alu.add, Alu.max
            )
        for t in range(16):
            if t == 5:
                flush_exp()
            for g in range(4):
                jp = 64 * bk + 16 * g + t
                if SKIP == "elem":
                    a = a_static
                else:
                    a = work.tile([128, 512], F16, name="a", tag="a")
                    if ACT_PAT == "tail":
                        # ACT owns the last column-group of each round (plus a
                        # few g=2 slots) so PE rounds never block mid-round.
                        use_act = (g == 3) or (g == 2 and t % 5 == 2)
                    else:
                        c = tile_state["ctr"]
                        use_act = ((c * ACT_N) // ACT_D) != (((c + 1) * ACT_N) // ACT_D)
                        tile_state["ctr"] = c + 1
                    if use_act:
                        nc.scalar.activation(
                            a[:], qt2[h][:], Act.Relu,
                            bias=ktp[h][:, jp : jp + 1], scale=1.0,
                        )
                    else:
                        nc.vector.tensor_scalar(
                            a[:], qt2[h][:], ktp[h][:, jp : jp + 1], 0.0,
                            Alu.add, Alu.max,
                        )
                if SKIP == "mm" and t > 0:
                    continue
                nc.tensor.matmul(
                    dt_ps[32 * g : 32 * g + 32, :],
                    sel[:, t, :], a[:],
                    start=(t == 0), stop=(SKIP == "mm"),
                    tile_position=(0, 32 * g),
                )
        if SKIP != "mm":
            for g in range(4):
                nc.tensor.matmul(
                    dt_ps[32 * g : 32 * g + 32, :],
                    ksr[h][0:1, 128 * bk + 32 * g : 128 * bk + 32 * g + 32],
                    onesrow[:],
                    start=False, stop=True,
                    tile_position=(0, 32 * g),
                )
        pending_exp.append((h, bk, dt_ps))

    emit_bank(0, 0)
    emit_bank(0, 1)
    emit_qkt(1)
    emit_bank(0, 2)
    emit_bank(0, 3)
    emit_bank(1, 0)
    v_sb = consts.tile([128, 4, 128], F16, name="v_sb")
    for jt in range(4):
        v_ps = paux.tile([128, 128], F32, name="v_ps", tag="aux")
        for kt in range(4):
            nc.tensor.matmul(
                v_ps[:], hidT[:, kt, jt * 128 : (jt + 1) * 128],
                wv2[:, kt, :],
                start=(kt == 0), stop=(kt == 3),
            )
        nc.vector.tensor_copy(v_sb[:, jt, :], v_ps[:])
    for bk in range(1, 4):
        emit_bank(1, bk)
    flush_exp()

    # ---- softmax normalize + AV per head ----
    normT = consts.tile([128, 512], F16, name="normT")
    for h in range(2):
        cs_ps = paux.tile([64, 512], F32, name="cs_ps", tag="aux")
        for jt in range(4):
            nc.tensor.matmul(
                cs_ps[:], ones64[:], et_sb[h][:, jt, :],
                start=(jt == 0), stop=(jt == 3),
            )
        recip = consts.tile([64, 512], F32, name=f"recip{h}")
        nc.vector.reciprocal(recip[:], cs_ps[:])
        av_ps = paux.tile([64, 512], F32, name="av_ps", tag="aux")
        for jt in range(4):
            nc.tensor.matmul(
                av_ps[:], v_sb[:, jt, 64 * h : 64 * h + 64],
                et_sb[h][:, jt, :],
                start=(jt == 0), stop=(jt == 3),
            )
        nc.vector.tensor_mul(normT[64 * h : 64 * h + 64, :], av_ps[:], recip[:])

    # ---- out-projection partial + store ----
    for st in range(4):
        f_ps = pbank.tile([128, 512], F32, name="f_ps", tag="bank")
        nc.tensor.matmul(
            f_ps[:], normT[:, st * 128 : (st + 1) * 128], wo2[:],
            start=True, stop=True,
        )
        o_sb = work.tile([128, 512], F32, name="o_sb", tag="o")
        if st % 2 == 0:
            nc.vector.tensor_copy(o_sb[:], f_ps[:])
        else:
            nc.scalar.copy(o_sb[:], f_ps[:])
        nc.sync.dma_start(outp_d[st * 128 : (st + 1) * 128, :], o_sb[:])



_NC = None


def _get_nc():
    global _NC
    if _NC is None:
        _NC = _build_program()
    return _NC


def _host_constants():
    sel = np.zeros((128, 16, 32), np.float16)
    for t in range(16):
        for p in range(128):
            sel[p, t, 2 * t + p // 64] = 2.0
    ones64 = np.ones((128, 64), np.float16)
    return sel, ones64


def kernel(hidden_states, Wq, bq, Wk, bk, Wv, bv, Wo, bo):
    hidden_states = np.asarray(hidden_states, np.float32)
    Wq, bq = np.asarray(Wq, np.float32), np.asarray(bq, np.float32)
    Wk, bk = np.asarray(Wk, np.float32), np.asarray(bk, np.float32)
    Wv, bv = np.asarray(Wv, np.float32), np.asarray(bv, np.float32)
    Wo, bo = np.asarray(Wo, np.float32), np.asarray(bo, np.float32)

    sel, ones64 = _host_constants()
    in_maps = []
    for core in range(N_CORES):
        b = core // 4
        hp = core % 4
        cols = slice(hp * 128, hp * 128 + 128)
        wk_sl = Wk[:, cols]
        bq_sl, bk_sl = bq[cols.start : cols.stop], bk[cols.start : cols.stop]
        bqcol = np.stack(
            [np.tile(bq_sl[lh * 64 : lh * 64 + 64], 2) for lh in range(2)], axis=1
        ).astype(np.float32)
        bkncol = np.stack(
            [np.tile(-bk_sl[lh * 64 : lh * 64 + 64], 2) for lh in range(2)], axis=1
        ).astype(np.float32)
        bks2 = np.array(
            [[bk_sl[0:64].sum(), bk_sl[64:128].sum()]], np.float32
        )
        in_maps.append(
            {
                "hidt": np.ascontiguousarray(hidden_states[b].T).astype(np.float16),
                "wq2": np.concatenate(
                    [
                        np.concatenate([Wq[:, cols][:, l * 64 : l * 64 + 64]] * 2, axis=1)
                        for l in range(2)
                    ],
                    axis=1,
                ).astype(np.float16),
                "wk2n": np.ascontiguousarray(-wk_sl).astype(np.float16),
                "wv2": np.ascontiguousarray(Wv[:, cols]).astype(np.float16),
                "wo2": np.ascontiguousarray(Wo[cols, :]).astype(np.float16),
                "wks2": wk_sl.reshape(HID, 2, 64).sum(-1).astype(np.float16),
                "bqcol": bqcol,
                "bkncol": bkncol,
                "bks2": bks2,
                "sel": sel,
                "ones64": ones64,
            }
        )

    nc = _get_nc()
    res = run_bass_kernel_spmd(nc, in_maps, core_ids=list(range(N_CORES)))
    parts = [r["outp"] for r in res.results]
    bo_eff = bv @ Wo + bo
    out = np.stack(
        [
            parts[0] + parts[1] + parts[2] + parts[3],
            parts[4] + parts[5] + parts[6] + parts[7],
        ],
        axis=0,
    )
    return (out + bo_eff[None, None, :]).astype(np.float32)


if __name__ == "__main__":
    rng = np.random.default_rng(0)
    w = 0.02
    inputs = {
        "hidden_states": rng.standard_normal((B, S, HID)).astype(np.float32),
        "Wq": (rng.standard_normal((HID, HID)) * w).astype(np.float32),
        "bq": np.zeros(HID, np.float32),
        "Wk": (rng.standard_normal((HID, HID)) * w).astype(np.float32),
        "bk": np.zeros(HID, np.float32),
        "Wv": (rng.standard_normal((HID, HID)) * w).astype(np.float32),
        "bv": np.zeros(HID, np.float32),
        "Wo": (rng.standard_normal((HID, HID)) * w).astype(np.float32),
        "bo": np.zeros(HID, np.float32),
    }
    out = kernel(**inputs)
    print("out shape:", out.shape, "finite:", np.isfinite(out).all())

